# revision 29
# baseline (speedup 1.0000x reference)
"""Causal multi-head attention (B=2, S=2048, D=768, H=12) on 8 Trainium2 cores.

Sharding: core c -> batch b = c//4, head-group g = c%4 (heads 3g..3g+2).
Each core computes its 3 heads end-to-end in bf16 (fp32 PSUM accumulation)
and produces a partial output-projection y_partial[b] = out_g @ Wo_g^T
(+ bo on g==0 cores).  The host sums the 4 partials per batch (the
"all-reduce") while unsharding.

Device layout notes (per core):
  qkT groups (dim-on-partition, token-on-free), each [128, 2048] bf16:
    G0 = [q_h0 (p0-63) ; q_h1 (p64-127)]
    G1 = [k_h0 ; k_h1]
    G2 = [q_h2 ; k_h2]
  q_h2 is DMA-rehomed to partitions 64-127 and k_h2 to partitions 0-63 so
  head-2 score matmuls can alternate between PE row-groups (load balance
  against heads 0/1 which are pinned to row groups 0 and 64).
  Scores are computed transposed S_T[k, q] so the softmax denominator
  falls out of the AV matmul via a ones column appended to v.  The
  denominator row is broadcast across partitions with a tiny ones-matmul,
  reciprocal'd on the DVE, and multiplied into the AV rows.
"""

import os
import sys

import numpy as np

for _p in ("/opt/trn_rl_repo",):
    if _p not in sys.path and os.path.isdir(_p):
        sys.path.insert(0, _p)

import ml_dtypes  # noqa: E402

import concourse.bass as bass  # noqa: E402
import concourse.mybir as mybir  # noqa: E402
import concourse.tile as tile  # noqa: E402
from concourse import bacc  # noqa: E402
from concourse.bass_utils import run_bass_kernel_spmd  # noqa: E402
from concourse.tile_rust import add_dep_helper  # noqa: E402

BF16 = mybir.dt.bfloat16
F32 = mybir.dt.float32
F8 = mybir.dt.float8e4
NPBF = ml_dtypes.bfloat16
NPF8 = ml_dtypes.float8_e4m3
WSCALE = 16.0  # host premultiplier on Wq/Wk so fp8 stays out of subnormals
ESHIFT = float(-np.log(16.0))  # exp bias: es scaled by 1/16 to fit fp8e4 range
DR = mybir.MatmulPerfMode.DoubleRow

B, S, D = 2, 2048, 768
H, HD = 12, 64
NCORE = 8
HPC = 3  # heads per core
FT = D // 128  # 6 contraction tiles for projections
ST = S // 128  # 16 token tiles
QC = S // 512  # 4 q-chunks of 512
SCALE = float(1.0 / np.sqrt(HD))

_CACHE: dict = {}


def _emit(nc: bacc.Bacc, tc: tile.TileContext, dr: dict, y_dr) -> None:
    from contextlib import ExitStack

    Exp = mybir.ActivationFunctionType.Exp

    with ExitStack() as ex:
        pool = lambda name, bufs, space="SBUF": ex.enter_context(  # noqa: E731
            tc.tile_pool(name=name, bufs=bufs, space=space)
        )

        consts = pool("consts", 1)

        # ---- persistent SBUF tensors -------------------------------------
        xT = consts.tile([128, QC, FT, 512], BF16)  # x[b]^T, chunk-major
        wqk = consts.tile([128, 3, FT, 128], BF16)  # qk projection lhsT tiles (g-major)
        wv = consts.tile([128, FT, 192], BF16)  # v projection rhs tiles
        woAB = consts.tile([128, D], BF16)  # out-proj rhs, heads 0+1 packed
        woC = consts.tile([128, D], BF16)  # out-proj rhs, head 2 (rows duplicated)
        bqk = consts.tile([128, 3], F32)
        bv = consts.tile([128, 192], F32)
        mask = consts.tile([128, 128], BF16)  # tri mask m[p,c]=1 if p<=c
        ones = consts.tile([65, 64], mybir.dt.float16)  # bcast matmul lhsT (row 64)
        ebias = consts.tile([128, 1], F32)  # exp bias (-ln16) per partition

        qkT = consts.tile([128, 3, S], BF16)  # projected q/k groups (16x scaled)
        qCmv = consts.tile([128, S], BF16)  # q_h2 rehomed to partitions 64-127
        kCmv = consts.tile([128, S], BF16)  # k_h2 rehomed to partitions 0-63
        vsb = consts.tile([128, ST, HPC, 65], BF16)  # v (+ones col) per ktile
        vsb8 = consts.tile([128, 12, HPC, 80], F8)  # fp8 v (+ones col), ktiles 0-11
        outAB = consts.tile([128, S], BF16)  # normalized out_T heads 0 (+1 moved)
        outC = consts.tile([128, S], BF16)  # normalized out_T head 2; even stiles
        # live on partitions 0-63, odd stiles on 64-127 so tail C-oprojs pair
        outB = consts.tile([64, S], BF16)  # normalized out_T head 1 (pre-move)

        # Input streaming.  Measured queue behavior: one queue's entries
        # pipeline at ~240GB/s; multiple queues share ~350-400GB/s of fabric.
        # The first projection chain needs wqk + x chunk 0, so that set is
        # split ACROSS queues (sync: wqk+x0a; gpsimd: x0b) to use the full
        # fabric, and the only early bulk competitor (scalar queue: woAB+xc2)
        # is issue-gated behind x0's completion.
        xview = dr["xT"].ap().rearrange("p (c f s) -> p c f s", c=QC, f=FT)
        wqkv = dr["wqk"].ap().rearrange("p (g f m) -> p g f m", g=3, f=FT)
        nc.sync.dma_start(out=wqk[:, 0], in_=wqkv[:, 0])
        nc.sync.dma_start(out=xT[:, 0, 0:3, :], in_=xview[:, 0, 0:3, :])
        d_x0b = nc.sync.dma_start(out=xT[:, 0, 3:FT, :], in_=xview[:, 0, 3:FT, :])
        nc.sync.dma_start(out=wqk[:, 1:3], in_=wqkv[:, 1:3])
        nc.sync.dma_start(out=xT[:, 1, :, :], in_=xview[:, 1, :, :])
        nc.sync.dma_start(out=xT[:, 3, :, :], in_=xview[:, 3, :, :])
        d_wv = nc.gpsimd.dma_start(
            out=wv[:], in_=dr["wv"].ap().rearrange("p (f m) -> p f m", f=FT)
        )
        add_dep_helper(
            d_wv.ins, d_x0b.ins, sync=True, reason="input priority: wv after x0"
        )
        nc.gpsimd.dma_start(out=woC[:], in_=dr["woC"].ap())
        nc.scalar.dma_start(out=bqk[:], in_=dr["bqk"].ap())
        nc.scalar.dma_start(out=mask[:], in_=dr["mask"].ap())
        nc.scalar.dma_start(out=bv[:], in_=dr["bv"].ap())
        d_woAB = nc.scalar.dma_start(out=woAB[:], in_=dr["woAB"].ap())
        add_dep_helper(
            d_woAB.ins, d_x0b.ins, sync=True, reason="input priority: woAB after x0"
        )
        nc.scalar.dma_start(out=xT[:, 2, :, :], in_=xview[:, 2, :, :])
        nc.vector.memset(vsb[:, :, :, 64:65], 1.0)
        nc.vector.memset(vsb8[:, :, :, 64:65], 1.0)
        nc.vector.memset(ones[64:65, :], 1.0)
        nc.vector.memset(ebias[:], ESHIFT)

        # ---- PSUM pools (8 banks total, statically allocated) -------------
        # ps_big: 2 slots x 2 banks  -> qk-proj, v-proj, scores(A,B), out-proj
        # ps_av : 3 slots x 1 bank   -> AV accumulators
        # ps_sm : 1 slot  x 1 bank   -> scores(C)
        ps_big = pool("ps_big", 2, "PSUM")
        ps_av = pool("ps_av", 3, "PSUM")
        ps_sm = pool("ps_sm", 1, "PSUM")

        # ---- projection pieces (emitted interleaved with attention) --------
        def emit_proj_qk(q4, g):
            qs = slice(512 * q4, 512 * (q4 + 1))
            ps = ps_big.tile([128, 1024], F32, tag="big", name=f"qkp_{g}_{q4}")
            for f in range(FT):
                nc.tensor.matmul(
                    ps[:, 0:512],
                    lhsT=wqk[:, g, f, :],
                    rhs=xT[:, q4, f, :],
                    start=(f == 0),
                    stop=(f == FT - 1),
                )
            nc.vector.tensor_scalar_add(qkT[:, g, qs], ps[:, 0:512], bqk[:, g : g + 1])
            if g == 2:
                # rehome head-2 q/k so C-scores can run on either row-group
                nc.sync.dma_start(out=qCmv[64:128, qs], in_=qkT[0:64, 2, qs])
                nc.sync.dma_start(out=kCmv[0:64, qs], in_=qkT[64:128, 2, qs])

        def emit_proj_v(st):
            ps = ps_big.tile([128, 1024], F32, tag="big", name=f"vp_{st}")
            c, sub = st // 4, st % 4
            for f in range(FT):
                nc.tensor.matmul(
                    ps[:, 0:192],
                    lhsT=xT[:, c, f, 128 * sub : 128 * (sub + 1)],
                    rhs=wv[:, f, :],
                    start=(f == 0),
                    stop=(f == FT - 1),
                )
            nc.vector.tensor_add(
                vsb[:, st, :, 0:64],
                ps[:, 0:192].rearrange("p (h d) -> p h d", h=3),
                bv[:].rearrange("p (h d) -> p h d", h=3),
            )
            if st < 12:  # fp8 copy for DoubleRow AV (off-diagonal ktiles only)
                nc.vector.tensor_copy(vsb8[:, st, :, 0:64], vsb[:, st, :, 0:64])

        def proj_pieces(c):
            out = [lambda g=g: emit_proj_qk(c, g) for g in range(3)]
            out += [lambda st=st: emit_proj_v(st) for st in range(4 * c, 4 * c + 4)]
            return out

        # ---- attention -----------------------------------------------------
        # es for off-diagonal ("full") steps lives in fp8 PAIR tiles (two
        # consecutive ktiles) so the AV can run as fp8 DoubleRow matmuls
        # contracting 256 k at once.  Diagonal steps stay bf16 (their queries
        # include short-prefix tokens where attention concentrates and fp8 v
        # noise would not average out).  ALL exps carry bias=-ln16 so fp8 es
        # stays in range; the 1/16 cancels in the softmax normalization.
        exp_sb = pool("exp_sb", 6)
        exp8_sb = pool("exp8_sb", 4)
        den_sb = pool("den_sb", 3)
        rec_sb = pool("rec_sb", 3)
        SCL = SCALE

        def emit_scores(j, i):
            full = i < 4 * j
            off = 0 if full else 128 * (i - 4 * j)
            qs = slice(512 * j + off, 512 * (j + 1))
            ks = slice(128 * i, 128 * (i + 1))
            sAB_raw = ps_big.tile([128, 1024], F32, tag="big", name=f"sAB_{j}_{i}")
            sAB = sAB_raw[:].rearrange("p (h q) -> p h q", h=2)
            sC = ps_sm.tile([128, 512], F32, tag="sm", name=f"sC_{j}_{i}")
            mmA = nc.tensor.matmul(
                sAB[:, 0, off:], lhsT=qkT[0:64, 1, ks], rhs=qkT[0:64, 0, qs]
            )
            if i == 0:
                gate.clear()
            if i < 2:
                gate.append(mmA)
            nc.tensor.matmul(
                sAB[:, 1, off:], lhsT=qkT[64:128, 1, ks], rhs=qkT[64:128, 0, qs]
            )
            if i % 2 == 0:
                nc.tensor.matmul(sC[:, off:], lhsT=kCmv[0:64, ks], rhs=qkT[0:64, 2, qs])
            else:
                nc.tensor.matmul(
                    sC[:, off:], lhsT=qkT[64:128, 2, ks], rhs=qCmv[64:128, qs]
                )
            if full:
                if i % 2 == 0:
                    es = exp8_sb.tile(
                        [128, 2, HPC, 512], F8, tag="es8", name=f"es8_{j}_{i}"
                    )
                    es_pair[(j, i + 1)] = es
                else:
                    es = es_pair.pop((j, i))
                sl = i % 2
                nc.scalar.activation(
                    es[:, sl, 2, :], sC[:, 0:512], Exp, scale=SCL, bias=ebias[:]
                )

                def emit_ab():
                    nc.scalar.activation(
                        es[:, sl, 0:2, :].rearrange("p h q -> p (h q)"),
                        sAB_raw[:, 0:1024],
                        Exp,
                        scale=SCL,
                        bias=ebias[:],
                    )

                return ("f8", es, sl), emit_ab
            es = exp_sb.tile([128, HPC, 512], BF16, tag="es", name=f"es_{j}_{i}")
            # exp C first: sC is single-buffered, so freeing it early keeps
            # the next step's C matmul off the ACT critical path (sAB has 2
            # slots and tolerates the extra lag)
            nc.scalar.activation(es[:, 2, off:], sC[:, off:], Exp, scale=SCL, bias=ebias[:])

            def emit_ab():
                if off == 0:
                    # contiguous fast path: flat 1-D APs for the A|B pair
                    nc.scalar.activation(
                        es[:].rearrange("p h q -> p (h q)")[:, 0:1024],
                        sAB_raw[:, 0:1024],
                        Exp,
                        scale=SCL,
                        bias=ebias[:],
                    )
                else:
                    nc.scalar.activation(
                        es[:, 0:2, off:], sAB[:, :, off:], Exp, scale=SCL, bias=ebias[:]
                    )

            return ("bf", es, off), emit_ab

        def emit_av(j, i, nk, esrec, av):
            kind, es, aux = esrec
            if kind == "f8":
                if i % 2 == 0:
                    return  # handled together with the odd pair member
                p = i - 1
                for h in range(HPC):
                    nc.tensor.matmul(
                        av[h][0:65, :],
                        lhsT=vsb8[:, p : p + 2, h, 0:65],
                        rhs=es[:, :, h, :],
                        start=(p == 0),
                        stop=False,
                        perf_mode=DR,
                    )
                return
            off = aux
            if i >= 4 * j:  # diagonal block: zero the k>q half
                dm = slice(off, off + 128)
                nc.vector.tensor_mul(
                    es[:, :, dm],
                    es[:, :, dm],
                    mask[:, None, :].broadcast_to([128, HPC, 128]),
                )
            for h in range(HPC):
                mm = nc.tensor.matmul(
                    av[h][0:65, off:],
                    lhsT=vsb[:, i, h, :],
                    rhs=es[:, h, off:],
                    start=(i == 0),
                    stop=(i == nk - 1),
                )
                # at the last chunk boundary, let its first scores beat the
                # prior chunk's AV backlog onto the PE stream (ordering only)
                if i >= nk - 3 and gate:
                    for gmm in gate:
                        add_dep_helper(
                            mm.ins, gmm.ins, sync=False,
                            reason="boundary: scores before AV backlog",
                        )

        def emit_norm(j, av):
            qs_full = slice(512 * j, 512 * (j + 1))
            # normalization: out = outU * (1/denom) ; denom = av row 64.
            # Stage-ordered (copies, broadcasts, recips, muls) so the first
            # AV slot frees as early as possible.
            dens = []
            for h in range(HPC):
                den = den_sb.tile(
                    [65, 512], mybir.dt.float16, tag="den", name=f"dn_{j}_{h}"
                )
                nc.vector.tensor_copy(den[64:65, 0:512], av[h][64:65, :])
                dens.append(den)
            # h2's broadcast keeps the single-slot sC pool (allocated first so
            # it frees fastest); h1/h0 share halves of one big-pool tile so
            # the sC pool isn't serialized behind the whole norm.  h1 before
            # h0: its result needs a rehome DMA before the packed out-proj.
            bigbc = ps_big.tile([128, 1024], F32, tag="big", name=f"bch_{j}")
            bc_ap = {
                2: ps_sm.tile([128, 512], F32, tag="sm", name=f"b_{j}_2"),
                1: bigbc[:, 0:512],
                0: bigbc[:, 512:1024],
            }
            for h in (2, 1, 0):
                bc = bc_ap[h]
                nc.tensor.matmul(
                    bc[0:64, :], lhsT=ones[64:65, :], rhs=dens[h][64:65, 0:512]
                )
                rec = rec_sb.tile([64, 512], F32, tag="rec", name=f"rc_{j}_{h}")
                nc.vector.reciprocal_approx_fast(rec[:], bc[0:64, :])
                dst = (outAB[0:64, qs_full], outB[:, qs_full], outC[0:64, qs_full])[h]
                nc.vector.tensor_mul(dst, av[h][0:64, :], rec[:])
                if h == 1:
                    # move head-1 slice onto partitions 64-127 for out-proj
                    nc.sync.dma_start(out=outAB[64:128, qs_full], in_=outB[:, qs_full])
                if h == 2:
                    # rehome ODD stiles' h2 rows to partitions 64-127 so the
                    # tail's C-oprojs can pair across PE row groups
                    odd = outC[0:64, qs_full].rearrange("p (a c) -> p a c", a=4)
                    oddhi = outC[64:128, qs_full].rearrange("p (a c) -> p a c", a=4)
                    nc.sync.dma_start(out=oddhi[:, 1], in_=odd[:, 1])
                    nc.sync.dma_start(out=oddhi[:, 3], in_=odd[:, 3])

        # y stages in a persistent SBUF buffer (partition-major DRAM layout so
        # per-chunk DMAs move 6KB runs; host re-lays out to [S, D])
        ybuf = consts.tile([128, ST, D], mybir.dt.float16)
        y_view = y_dr.ap().rearrange("p (c k d) -> p c (k d)", c=QC, k=4)
        yh_view = y_dr.ap().rearrange("p (hh d) -> p hh d", hh=8)

        def emit_oproj_ab(st):
            ss = slice(128 * st, 128 * (st + 1))
            ps = ps_big.tile([128, 1024], F32, tag="big", name=f"yp_{st}")
            for n0, nw in ((0, 512), (512, 256)):
                nc.tensor.matmul(
                    ps[:, n0 : n0 + nw],
                    lhsT=outAB[:, ss],
                    rhs=woAB[:, n0 : n0 + nw],
                    start=True,
                    stop=False,
                )
            return ps

        def emit_oproj_c(st, ps):
            ss = slice(128 * st, 128 * (st + 1))
            rows = slice(64 * (st % 2), 64 * (st % 2) + 64)
            for n0, nw in ((0, 512), (512, 256)):
                nc.tensor.matmul(
                    ps[:, n0 : n0 + nw],
                    lhsT=outC[rows, ss],
                    rhs=woC[rows, n0 : n0 + nw],
                    start=False,
                    stop=True,
                )

        def emit_oproj(st):
            # both AB matmuls first, then both C: the stationary weights
            # change once per stile instead of three times (each PSUM region
            # still gets its own start->stop accumulation pair)
            ps = emit_oproj_ab(st)
            emit_oproj_c(st, ps)
            emit_oproj_fin(st, ps)

        def emit_oproj_fin(st, ps):
            # fp16 staging copy: DVE while ACT is busy with exp; in the last
            # chunk (exp winding down) alternate onto ACT so the tail's copies
            # don't serialize on one engine
            # stiles 8-15 run post-exp (OP_FROM=3 pops only 0-7 mid-chunk-3),
            # where ACT is idle and 8 serialized DVE copies (~7.6us) would be
            # the tail's critical path — alternate DVE/ACT there
            if st >= TAILALT and st % 2 == 1:
                nc.scalar.copy(ybuf[:, st, :], ps[:, 0:D])
            else:
                nc.vector.tensor_copy(ybuf[:, st, :], ps[:, 0:D])
            if st == 13:
                # last chunk: split across both HWDGE queues so the final y
                # bytes overlap the remaining out-proj matmuls (safe here:
                # ACT's exp stream is done, so its issue-wait can't stall exp)
                nc.scalar.dma_start(
                    out=yh_view[:, 6],
                    in_=ybuf[:, 12:14, :].rearrange("p k d -> p (k d)"),
                )
            elif st == 15:
                nc.sync.dma_start(
                    out=yh_view[:, 7],
                    in_=ybuf[:, 14:16, :].rearrange("p k d -> p (k d)"),
                )
            elif st % 4 == 3:
                c = st // 4
                nc.sync.dma_start(
                    out=y_view[:, c],
                    in_=ybuf[:, 4 * c : 4 * c + 4, :].rearrange("p k d -> p (k d)"),
                )

        # flat software pipeline over all (j, i) steps: scores/exp run LAG
        # steps ahead of AV, crossing chunk boundaries so neither PE nor ACT
        # drains at chunk turns.  Norms are delayed NDELAY further steps so
        # their PE broadcast matmuls never gate the scores stream, and each
        # chunk's out-projection is queued behind the following chunk's AV
        # (it steals "av" PSUM slots, so it runs once those free up).
        LAG = int(os.environ.get("LAG", "4"))
        NDELAY = int(os.environ.get("NDELAY", "0"))
        # out-projections drain into the following chunks' step stream (one
        # stile every OP_EVERY steps) so y DMA streams throughout the kernel
        # instead of piling into a tail; 0 disables (all at the end).
        OP_EARLY = int(os.environ.get("OP_EARLY", "2"))
        OP_FROM = int(os.environ.get("OP_FROM", "3"))  # first chunk that interleaves oproj
        steps = [(j, i) for j in range(QC) for i in range(4 * (j + 1))]
        av_of: dict = {}
        es_of: dict = {}
        es_pair: dict = {}
        gate: list = []
        work_q: list = []  # deferred (fn, args) emissions
        pend_oproj: list = []

        def do_av(idx):
            pj, pi = steps[idx]
            nkp = 4 * (pj + 1)
            if pi == 0:
                av_of[pj] = [
                    ps_av.tile([128, 512], F32, tag="av", name=f"av_{pj}_{h}")
                    for h in range(HPC)
                ]
            emit_av(pj, pi, nkp, es_of.pop((pj, pi)), av_of[pj])
            if pi == nkp - 1:
                work_q.append(("norm", pj, NDELAY))

        def drain_work_q():
            rest = []
            for kind, arg, delay in work_q:
                if delay > 0:
                    rest.append((kind, arg, delay - 1))
                    continue
                if kind == "norm":
                    emit_norm(arg, av_of.pop(arg))
                    if OP_EARLY:
                        pend_oproj.extend(range(4 * arg, 4 * arg + 4))
            work_q[:] = rest

        # proj chunk c+1's pieces are spread across attention chunk c's steps
        # (attention chunk j only needs projection chunks <= j).
        # PE pipeline refills (~160ns) are paid at every matmul SHAPE switch
        # (scores (64,128) / AV (128,65) / proj+oproj (128,128)), so emission
        # batches same-shape work: AVs in PAIR-step bursts, proj pieces at
        # 4-step boundaries.
        PAIR = int(os.environ.get("PAIR", "2"))
        AB_DEFER = int(os.environ.get("AB_DEFER", "0"))
        OP_PHASE = int(os.environ.get("OP_PHASE", "0"))
        TAILALT = int(os.environ.get("TAILALT", "12"))
        pend_ab: list = []
        for piece in proj_pieces(0):
            piece()
        pend_proj: list = list(proj_pieces(1))
        for idx, (j, i) in enumerate(steps):
            nk = 4 * (j + 1)
            if pend_proj:
                quota = max(1, -(-len(pend_proj) // max(1, nk - i)))
                for _ in range(quota):
                    if pend_proj:
                        pend_proj.pop(0)()
            if i == nk - 1 and j + 2 < QC:
                pend_proj = list(proj_pieces(j + 2))
            es_of[(j, i)], ab_fn = emit_scores(j, i)
            # defer each step's AB exp by AB_DEFER steps on the ACT stream:
            # the single-buffered C path gains a step of slack (its expC no
            # longer queues behind the previous step's big AB exp)
            pend_ab.append(ab_fn)
            while len(pend_ab) > AB_DEFER:
                pend_ab.pop(0)()
            if idx >= LAG and (idx - LAG) % PAIR == PAIR - 1:
                for k in range(PAIR):
                    do_av(idx - LAG - PAIR + 1 + k)
            drain_work_q()
            if OP_EARLY and pend_oproj and j >= OP_FROM and i % OP_EARLY == OP_PHASE:
                emit_oproj(pend_oproj.pop(0))
        while pend_ab:
            pend_ab.pop(0)()
        done_av = len(steps) - LAG
        done_av -= (done_av % PAIR)
        for idx in range(done_av, len(steps)):
            do_av(idx)
            drain_work_q()
        while work_q:
            drain_work_q()
        # tail drain: emit stile PAIRS so the two 64-row C matmuls (even stile
        # on PE rows 0-63, odd on 64-127) run concurrently
        rest = list(pend_oproj) if OP_EARLY else list(range(ST))
        while rest:
            if len(rest) >= 2 and rest[0] % 2 != rest[1] % 2:
                s0, s1 = rest.pop(0), rest.pop(0)
                ps0 = emit_oproj_ab(s0)
                ps1 = emit_oproj_ab(s1)
                emit_oproj_c(s0, ps0)
                emit_oproj_c(s1, ps1)
                emit_oproj_fin(s0, ps0)
                emit_oproj_fin(s1, ps1)
            else:
                emit_oproj(rest.pop(0))


def _build():
    if "nc" in _CACHE:
        return _CACHE["nc"]
    nc = bacc.Bacc("TRN2", target_bir_lowering=False, debug=False, num_devices=NCORE)
    dr = {
        "xT": nc.dram_tensor("xT", [128, FT * S], BF16, kind="ExternalInput"),
        "wqk": nc.dram_tensor("wqk", [128, FT * 3 * 128], BF16, kind="ExternalInput"),
        "wv": nc.dram_tensor("wv", [128, FT * 192], BF16, kind="ExternalInput"),
        "woAB": nc.dram_tensor("woAB", [128, D], BF16, kind="ExternalInput"),
        "woC": nc.dram_tensor("woC", [128, D], BF16, kind="ExternalInput"),
        "bqk": nc.dram_tensor("bqk", [128, 3], F32, kind="ExternalInput"),
        "bv": nc.dram_tensor("bv", [128, 192], F32, kind="ExternalInput"),
        "mask": nc.dram_tensor("mask", [128, 128], BF16, kind="ExternalInput"),
    }
    y_dr = nc.dram_tensor("y", [128, ST * D], mybir.dt.float16, kind="ExternalOutput")
    with tile.TileContext(nc) as tc:
        _emit(nc, tc, dr, y_dr)
    nc.compile()
    _CACHE["nc"] = nc
    return nc


def prep_inputs(x, Wq, bq, Wk, bk, Wv, bv, Wo, bo):
    """Shard + pre-layout the full fp32 inputs into 8 per-core input maps."""
    in_maps = []
    mask = (np.arange(128)[:, None] <= np.arange(128)[None, :]).astype(NPBF)
    for c in range(NCORE):
        b, g = c // 4, c % 4
        hs = [3 * g, 3 * g + 1, 3 * g + 2]

        xT = np.ascontiguousarray(
            x[b].T.reshape(FT, 128, QC, 512).transpose(1, 2, 0, 3)
        )  # [128, QC, FT, 512] chunk-major

        def rows(W, h):
            return W[h * 64 : (h + 1) * 64]  # [64, D]

        G0 = np.concatenate([rows(Wq, hs[0]), rows(Wq, hs[1])], 0)  # [128, D]
        G1 = np.concatenate([rows(Wk, hs[0]), rows(Wk, hs[1])], 0)
        G2 = np.concatenate([rows(Wq, hs[2]), rows(Wk, hs[2])], 0)
        # wqk[p, g, f, m] = G_g[m, f*128+p]  (g-major so group 0 streams first)
        wqk = np.stack([G0, G1, G2], 0).transpose(2, 0, 1)  # [D, 3, 128]
        wqk = wqk.reshape(FT, 128, 3, 128).transpose(1, 2, 0, 3)  # [128, 3, FT, 128]

        Vg = Wv[g * 192 : (g + 1) * 192]  # [192, D]
        wv_ = Vg.T.reshape(FT, 128, 192).transpose(1, 0, 2)  # [128, FT, 192]

        # out-proj rhs: rows = local head dims, cols = output features
        woAB = np.concatenate(
            [
                Wo[:, (3 * g + 0) * 64 : (3 * g + 1) * 64].T,
                Wo[:, (3 * g + 1) * 64 : (3 * g + 2) * 64].T,
            ],
            0,
        )  # [128, D]
        woC = np.zeros((128, D), np.float32)
        woC[0:64] = Wo[:, (3 * g + 2) * 64 : (3 * g + 3) * 64].T
        woC[64:128] = woC[0:64]  # duplicated so odd stiles read rows 64-127

        bqk_ = np.stack(
            [
                np.concatenate([bq[hs[0] * 64 : hs[0] * 64 + 64], bq[hs[1] * 64 : hs[1] * 64 + 64]]),
                np.concatenate([bk[hs[0] * 64 : hs[0] * 64 + 64], bk[hs[1] * 64 : hs[1] * 64 + 64]]),
                np.concatenate([bq[hs[2] * 64 : hs[2] * 64 + 64], bk[hs[2] * 64 : hs[2] * 64 + 64]]),
            ],
            1,
        ).astype(np.float32)  # [128, 3]

        bv_ = np.tile(bv[g * 192 : (g + 1) * 192][None, :], (128, 1)).astype(np.float32)

        in_maps.append(
            {
                "xT": xT.reshape(128, FT * S).astype(NPBF),
                "wqk": wqk.reshape(128, FT * 3 * 128).astype(NPBF),
                "wv": wv_.reshape(128, FT * 192).astype(NPBF),
                "woAB": woAB.astype(NPBF),
                "woC": woC.astype(NPBF),
                "bqk": bqk_,
                "bv": bv_,
                "mask": mask,
            }
        )
    return in_maps


def run_spmd(in_maps, trace=False, **kw):
    nc = _build()
    return run_bass_kernel_spmd(nc, in_maps, core_ids=list(range(NCORE)), trace=trace, **kw)


def gather(results, bo=None):
    y = np.zeros((B, S, D), np.float32)
    for c in range(NCORE):
        # y dram is [128, ST, D] partition-major: y_full[st*128+p] = y[p, st]
        yp = np.asarray(results[c]["y"], np.float32).reshape(128, ST, D)
        y[c // 4] += yp.transpose(1, 0, 2).reshape(S, D)
    if bo is not None:
        y += np.asarray(bo, np.float32)
    return y


def kernel(x, Wq, bq, Wk, bk, Wv, bv, Wo, bo):
    args = [np.asarray(a, np.float32) for a in (x, Wq, bq, Wk, bk, Wv, bv, Wo, bo)]
    in_maps = prep_inputs(*args)
    last_err = None
    for _attempt in range(3):
        try:
            res = run_spmd(in_maps)
            return gather(res.results, bo=args[8])
        except Exception as e:  # transient NRT/axon hiccups: retry
            last_err = e
            import time

            time.sleep(2.0)
    raise last_err



# revision 30
# speedup vs baseline: 1.2136x; 1.2136x over previous
"""Causal multi-head attention (B=2, S=2048, D=768, H=12) on 8 Trainium2 cores.

Sharding: core c -> batch b = c//4, head-group g = c%4 (heads 3g..3g+2).
Each core computes its 3 heads end-to-end in bf16 (fp32 PSUM accumulation)
and produces a partial output-projection y_partial[b] = out_g @ Wo_g^T
(+ bo on g==0 cores).  The host sums the 4 partials per batch (the
"all-reduce") while unsharding.

Device layout notes (per core):
  qkT groups (dim-on-partition, token-on-free), each [128, 2048] bf16:
    G0 = [q_h0 (p0-63) ; q_h1 (p64-127)]
    G1 = [k_h0 ; k_h1]
    G2 = [q_h2 ; k_h2]
  q_h2 is DMA-rehomed to partitions 64-127 and k_h2 to partitions 0-63 so
  head-2 score matmuls can alternate between PE row-groups (load balance
  against heads 0/1 which are pinned to row groups 0 and 64).
  Scores are computed transposed S_T[k, q] so the softmax denominator
  falls out of the AV matmul via a ones column appended to v.  The
  denominator row is broadcast across partitions with a tiny ones-matmul,
  reciprocal'd on the DVE, and multiplied into the AV rows.
"""

import os
import sys

import numpy as np

for _p in ("/opt/trn_rl_repo",):
    if _p not in sys.path and os.path.isdir(_p):
        sys.path.insert(0, _p)

import ml_dtypes  # noqa: E402

import concourse.bass as bass  # noqa: E402
import concourse.mybir as mybir  # noqa: E402
import concourse.tile as tile  # noqa: E402
from concourse import bacc  # noqa: E402
from concourse.bass_utils import run_bass_kernel_spmd  # noqa: E402
from concourse.tile_rust import add_dep_helper  # noqa: E402

BF16 = mybir.dt.bfloat16
F32 = mybir.dt.float32
F8 = mybir.dt.float8e4
NPBF = ml_dtypes.bfloat16
NPF8 = ml_dtypes.float8_e4m3
WSCALE = 16.0  # host premultiplier on Wq/Wk so fp8 stays out of subnormals
ESHIFT = float(-np.log(16.0))  # exp bias: es scaled by 1/16 to fit fp8e4 range
DR = mybir.MatmulPerfMode.DoubleRow

B, S, D = 2, 2048, 768
H, HD = 12, 64
NCORE = 8
HPC = 3  # heads per core
FT = D // 128  # 6 contraction tiles for projections
ST = S // 128  # 16 token tiles
QC = S // 512  # 4 q-chunks of 512
SCALE = float(1.0 / np.sqrt(HD))

_CACHE: dict = {}


def _emit(nc: bacc.Bacc, tc: tile.TileContext, dr: dict, y_dr) -> None:
    from contextlib import ExitStack

    Exp = mybir.ActivationFunctionType.Exp

    with ExitStack() as ex:
        pool = lambda name, bufs, space="SBUF": ex.enter_context(  # noqa: E731
            tc.tile_pool(name=name, bufs=bufs, space=space)
        )

        consts = pool("consts", 1)

        # ---- persistent SBUF tensors -------------------------------------
        xT = consts.tile([128, QC, FT, 512], BF16)  # x[b]^T, chunk-major
        wqk = consts.tile([128, 3, FT, 128], BF16)  # qk projection lhsT tiles (g-major)
        wv = consts.tile([128, FT, 192], BF16)  # v projection rhs tiles
        woAB = consts.tile([128, D], BF16)  # out-proj rhs, heads 0+1 packed
        woC = consts.tile([128, D], BF16)  # out-proj rhs, head 2 (rows duplicated)
        bqk = consts.tile([128, 3], F32)
        bv = consts.tile([128, 192], F32)
        mask = consts.tile([128, 128], BF16)  # tri mask m[p,c]=1 if p<=c
        ones = consts.tile([65, 64], mybir.dt.float16)  # bcast matmul lhsT (row 64)
        ebias = consts.tile([128, 1], F32)  # exp bias (-ln16) per partition

        qkT = consts.tile([128, 3, S], BF16)  # projected q/k groups (16x scaled)
        qCmv = consts.tile([128, S], BF16)  # q_h2 rehomed to partitions 64-127
        kCmv = consts.tile([128, S], BF16)  # k_h2 rehomed to partitions 0-63
        vsb = consts.tile([128, ST, HPC, 65], BF16)  # v (+ones col) per ktile
        vsb8 = consts.tile([128, 12, HPC, 80], F8)  # fp8 v (+ones col), ktiles 0-11
        outAB = consts.tile([128, S], BF16)  # normalized out_T heads 0 (+1 moved)
        outC = consts.tile([128, S], BF16)  # normalized out_T head 2; even stiles
        # live on partitions 0-63, odd stiles on 64-127 so tail C-oprojs pair
        outB = consts.tile([64, S], BF16)  # normalized out_T head 1 (pre-move)

        # Input streaming.  Measured queue behavior: one queue's entries
        # pipeline at ~240GB/s; multiple queues share ~350-400GB/s of fabric.
        # The first projection chain needs wqk + x chunk 0, so that set is
        # split ACROSS queues (sync: wqk+x0a; gpsimd: x0b) to use the full
        # fabric, and the only early bulk competitor (scalar queue: woAB+xc2)
        # is issue-gated behind x0's completion.
        xview = dr["xT"].ap().rearrange("p (c f s) -> p c f s", c=QC, f=FT)
        wqkv = dr["wqk"].ap().rearrange("p (g f m) -> p g f m", g=3, f=FT)
        nc.sync.dma_start(out=wqk[:, 0], in_=wqkv[:, 0])
        nc.sync.dma_start(out=xT[:, 0, 0:3, :], in_=xview[:, 0, 0:3, :])
        d_x0b = nc.sync.dma_start(out=xT[:, 0, 3:FT, :], in_=xview[:, 0, 3:FT, :])
        nc.sync.dma_start(out=wqk[:, 1:3], in_=wqkv[:, 1:3])
        nc.sync.dma_start(out=xT[:, 1, :, :], in_=xview[:, 1, :, :])
        nc.sync.dma_start(out=xT[:, 3, :, :], in_=xview[:, 3, :, :])
        d_wv = nc.gpsimd.dma_start(
            out=wv[:], in_=dr["wv"].ap().rearrange("p (f m) -> p f m", f=FT)
        )
        add_dep_helper(
            d_wv.ins, d_x0b.ins, sync=True, reason="input priority: wv after x0"
        )
        nc.gpsimd.dma_start(out=woC[:], in_=dr["woC"].ap())
        nc.scalar.dma_start(out=bqk[:], in_=dr["bqk"].ap())
        nc.scalar.dma_start(out=mask[:], in_=dr["mask"].ap())
        nc.scalar.dma_start(out=bv[:], in_=dr["bv"].ap())
        d_woAB = nc.scalar.dma_start(out=woAB[:], in_=dr["woAB"].ap())
        add_dep_helper(
            d_woAB.ins, d_x0b.ins, sync=True, reason="input priority: woAB after x0"
        )
        nc.scalar.dma_start(out=xT[:, 2, :, :], in_=xview[:, 2, :, :])
        nc.vector.memset(vsb[:, :, :, 64:65], 1.0)
        nc.vector.memset(vsb8[:, :, :, 64:65], 1.0)
        nc.vector.memset(ones[64:65, :], 1.0)
        nc.vector.memset(ebias[:], ESHIFT)

        # ---- PSUM pools (8 banks total, statically allocated) -------------
        # ps_big: 2 slots x 2 banks  -> qk-proj, v-proj, scores(A,B), out-proj
        # ps_av : 3 slots x 1 bank   -> AV accumulators
        # ps_sm : 1 slot  x 1 bank   -> scores(C)
        ps_big = pool("ps_big", 2, "PSUM")
        ps_av = pool("ps_av", 3, "PSUM")
        ps_sm = pool("ps_sm", 1, "PSUM")

        # ---- projection pieces (emitted interleaved with attention) --------
        def emit_proj_qk(q4, g):
            qs = slice(512 * q4, 512 * (q4 + 1))
            ps = ps_big.tile([128, 1024], F32, tag="big", name=f"qkp_{g}_{q4}")
            for f in range(FT):
                nc.tensor.matmul(
                    ps[:, 0:512],
                    lhsT=wqk[:, g, f, :],
                    rhs=xT[:, q4, f, :],
                    start=(f == 0),
                    stop=(f == FT - 1),
                )
            nc.vector.tensor_scalar_add(qkT[:, g, qs], ps[:, 0:512], bqk[:, g : g + 1])
            if g == 2:
                # rehome head-2 q/k so C-scores can run on either row-group
                nc.sync.dma_start(out=qCmv[64:128, qs], in_=qkT[0:64, 2, qs])
                nc.sync.dma_start(out=kCmv[0:64, qs], in_=qkT[64:128, 2, qs])

        def emit_proj_v(st):
            ps = ps_big.tile([128, 1024], F32, tag="big", name=f"vp_{st}")
            c, sub = st // 4, st % 4
            for f in range(FT):
                nc.tensor.matmul(
                    ps[:, 0:192],
                    lhsT=xT[:, c, f, 128 * sub : 128 * (sub + 1)],
                    rhs=wv[:, f, :],
                    start=(f == 0),
                    stop=(f == FT - 1),
                )
            nc.vector.tensor_add(
                vsb[:, st, :, 0:64],
                ps[:, 0:192].rearrange("p (h d) -> p h d", h=3),
                bv[:].rearrange("p (h d) -> p h d", h=3),
            )
            if st < 12:  # fp8 copy for DoubleRow AV (off-diagonal ktiles only)
                nc.vector.tensor_copy(vsb8[:, st, :, 0:64], vsb[:, st, :, 0:64])

        def proj_pieces(c):
            out = [lambda g=g: emit_proj_qk(c, g) for g in range(3)]
            out += [lambda st=st: emit_proj_v(st) for st in range(4 * c, 4 * c + 4)]
            return out

        # ---- attention -----------------------------------------------------
        # es for off-diagonal ("full") steps lives in fp8 PAIR tiles (two
        # consecutive ktiles) so the AV can run as fp8 DoubleRow matmuls
        # contracting 256 k at once.  Diagonal steps stay bf16 (their queries
        # include short-prefix tokens where attention concentrates and fp8 v
        # noise would not average out).  ALL exps carry bias=-ln16 so fp8 es
        # stays in range; the 1/16 cancels in the softmax normalization.
        exp_sb = pool("exp_sb", 6)
        exp8_sb = pool("exp8_sb", 4)
        den_sb = pool("den_sb", 3)
        rec_sb = pool("rec_sb", 3)
        SCL = SCALE

        def emit_scores(j, i):
            full = i < 4 * j
            off = 0 if full else 128 * (i - 4 * j)
            qs = slice(512 * j + off, 512 * (j + 1))
            ks = slice(128 * i, 128 * (i + 1))
            sAB_raw = ps_big.tile([128, 1024], F32, tag="big", name=f"sAB_{j}_{i}")
            sAB = sAB_raw[:].rearrange("p (h q) -> p h q", h=2)
            sC = ps_sm.tile([128, 512], F32, tag="sm", name=f"sC_{j}_{i}")
            mmA = nc.tensor.matmul(
                sAB[:, 0, off:], lhsT=qkT[0:64, 1, ks], rhs=qkT[0:64, 0, qs]
            )
            if i == 0:
                gate.clear()
            if i < 2:
                gate.append(mmA)
            nc.tensor.matmul(
                sAB[:, 1, off:], lhsT=qkT[64:128, 1, ks], rhs=qkT[64:128, 0, qs]
            )
            if i % 2 == 0:
                nc.tensor.matmul(sC[:, off:], lhsT=kCmv[0:64, ks], rhs=qkT[0:64, 2, qs])
            else:
                nc.tensor.matmul(
                    sC[:, off:], lhsT=qkT[64:128, 2, ks], rhs=qCmv[64:128, qs]
                )
            if full:
                if i % 2 == 0:
                    es = exp8_sb.tile(
                        [128, 2, HPC, 512], F8, tag="es8", name=f"es8_{j}_{i}"
                    )
                    es_pair[(j, i + 1)] = es
                else:
                    es = es_pair.pop((j, i))
                sl = i % 2
                nc.scalar.activation(
                    es[:, sl, 2, :], sC[:, 0:512], Exp, scale=SCL, bias=ebias[:]
                )

                def emit_ab():
                    nc.scalar.activation(
                        es[:, sl, 0:2, :].rearrange("p h q -> p (h q)"),
                        sAB_raw[:, 0:1024],
                        Exp,
                        scale=SCL,
                        bias=ebias[:],
                    )

                return ("f8", es, sl), emit_ab
            es = exp_sb.tile([128, HPC, 512], BF16, tag="es", name=f"es_{j}_{i}")
            # exp C first: sC is single-buffered, so freeing it early keeps
            # the next step's C matmul off the ACT critical path (sAB has 2
            # slots and tolerates the extra lag)
            nc.scalar.activation(es[:, 2, off:], sC[:, off:], Exp, scale=SCL, bias=ebias[:])

            def emit_ab():
                if off == 0:
                    # contiguous fast path: flat 1-D APs for the A|B pair
                    nc.scalar.activation(
                        es[:].rearrange("p h q -> p (h q)")[:, 0:1024],
                        sAB_raw[:, 0:1024],
                        Exp,
                        scale=SCL,
                        bias=ebias[:],
                    )
                else:
                    nc.scalar.activation(
                        es[:, 0:2, off:], sAB[:, :, off:], Exp, scale=SCL, bias=ebias[:]
                    )

            return ("bf", es, off), emit_ab

        def emit_av(j, i, nk, esrec, av):
            kind, es, aux = esrec
            if kind == "f8":
                if i % 2 == 0:
                    return  # handled together with the odd pair member
                p = i - 1
                for h in range(HPC):
                    nc.tensor.matmul(
                        av[h][0:65, :],
                        lhsT=vsb8[:, p : p + 2, h, 0:65],
                        rhs=es[:, :, h, :],
                        start=(p == 0),
                        stop=False,
                        perf_mode=DR,
                    )
                return
            off = aux
            if i >= 4 * j:  # diagonal block: zero the k>q half
                dm = slice(off, off + 128)
                nc.vector.tensor_mul(
                    es[:, :, dm],
                    es[:, :, dm],
                    mask[:, None, :].broadcast_to([128, HPC, 128]),
                )
            for h in range(HPC):
                mm = nc.tensor.matmul(
                    av[h][0:65, off:],
                    lhsT=vsb[:, i, h, :],
                    rhs=es[:, h, off:],
                    start=(i == 0),
                    stop=(i == nk - 1),
                )
                # at the last chunk boundary, let its first scores beat the
                # prior chunk's AV backlog onto the PE stream (ordering only)
                if i >= nk - 3 and gate:
                    for gmm in gate:
                        add_dep_helper(
                            mm.ins, gmm.ins, sync=False,
                            reason="boundary: scores before AV backlog",
                        )

        def emit_norm(j, av):
            qs_full = slice(512 * j, 512 * (j + 1))
            # normalization: out = outU * (1/denom) ; denom = av row 64.
            # Stage-ordered (copies, broadcasts, recips, muls) so the first
            # AV slot frees as early as possible.
            dens = []
            for h in range(HPC):
                den = den_sb.tile(
                    [65, 512], mybir.dt.float16, tag="den", name=f"dn_{j}_{h}"
                )
                nc.vector.tensor_copy(den[64:65, 0:512], av[h][64:65, :])
                dens.append(den)
            # h2's broadcast keeps the single-slot sC pool (allocated first so
            # it frees fastest); h1/h0 share halves of one big-pool tile so
            # the sC pool isn't serialized behind the whole norm.  h1 before
            # h0: its result needs a rehome DMA before the packed out-proj.
            bigbc = ps_big.tile([128, 1024], F32, tag="big", name=f"bch_{j}")
            bc_ap = {
                2: ps_sm.tile([128, 512], F32, tag="sm", name=f"b_{j}_2"),
                1: bigbc[:, 0:512],
                0: bigbc[:, 512:1024],
            }
            for h in (2, 1, 0):
                bc = bc_ap[h]
                nc.tensor.matmul(
                    bc[0:64, :], lhsT=ones[64:65, :], rhs=dens[h][64:65, 0:512]
                )
                rec = rec_sb.tile([64, 512], F32, tag="rec", name=f"rc_{j}_{h}")
                nc.vector.reciprocal_approx_fast(rec[:], bc[0:64, :])
                dst = (outAB[0:64, qs_full], outB[:, qs_full], outC[0:64, qs_full])[h]
                nc.vector.tensor_mul(dst, av[h][0:64, :], rec[:])
                if h == 1:
                    # move head-1 slice onto partitions 64-127 for out-proj
                    nc.sync.dma_start(out=outAB[64:128, qs_full], in_=outB[:, qs_full])
                if h == 2:
                    # rehome ODD stiles' h2 rows to partitions 64-127 so the
                    # tail's C-oprojs can pair across PE row groups
                    odd = outC[0:64, qs_full].rearrange("p (a c) -> p a c", a=4)
                    oddhi = outC[64:128, qs_full].rearrange("p (a c) -> p a c", a=4)
                    nc.sync.dma_start(out=oddhi[:, 1], in_=odd[:, 1])
                    nc.sync.dma_start(out=oddhi[:, 3], in_=odd[:, 3])

        # y stages in a persistent SBUF buffer (partition-major DRAM layout so
        # per-chunk DMAs move 6KB runs; host re-lays out to [S, D])
        ybuf = consts.tile([128, ST, D], mybir.dt.float16)
        y_view = y_dr.ap().rearrange("p (c k d) -> p c (k d)", c=QC, k=4)
        yh_view = y_dr.ap().rearrange("p (hh d) -> p hh d", hh=8)

        def emit_oproj_ab(st):
            ss = slice(128 * st, 128 * (st + 1))
            ps = ps_big.tile([128, 1024], F32, tag="big", name=f"yp_{st}")
            for n0, nw in ((0, 512), (512, 256)):
                nc.tensor.matmul(
                    ps[:, n0 : n0 + nw],
                    lhsT=outAB[:, ss],
                    rhs=woAB[:, n0 : n0 + nw],
                    start=True,
                    stop=False,
                )
            return ps

        def emit_oproj_c(st, ps):
            ss = slice(128 * st, 128 * (st + 1))
            rows = slice(64 * (st % 2), 64 * (st % 2) + 64)
            for n0, nw in ((0, 512), (512, 256)):
                nc.tensor.matmul(
                    ps[:, n0 : n0 + nw],
                    lhsT=outC[rows, ss],
                    rhs=woC[rows, n0 : n0 + nw],
                    start=False,
                    stop=True,
                )

        def emit_oproj(st):
            # both AB matmuls first, then both C: the stationary weights
            # change once per stile instead of three times (each PSUM region
            # still gets its own start->stop accumulation pair)
            ps = emit_oproj_ab(st)
            emit_oproj_c(st, ps)
            emit_oproj_fin(st, ps)

        def emit_oproj_fin(st, ps):
            # fp16 staging copy: DVE while ACT is busy with exp; in the last
            # chunk (exp winding down) alternate onto ACT so the tail's copies
            # don't serialize on one engine
            # stiles 8-15 run post-exp (OP_FROM=3 pops only 0-7 mid-chunk-3),
            # where ACT is idle and 8 serialized DVE copies (~7.6us) would be
            # the tail's critical path — alternate DVE/ACT there
            if st >= TAILALT and st % 2 == 1:
                nc.scalar.copy(ybuf[:, st, :], ps[:, 0:D])
            else:
                nc.vector.tensor_copy(ybuf[:, st, :], ps[:, 0:D])
            if st == 13:
                # last chunk: split across both HWDGE queues so the final y
                # bytes overlap the remaining out-proj matmuls (safe here:
                # ACT's exp stream is done, so its issue-wait can't stall exp)
                nc.scalar.dma_start(
                    out=yh_view[:, 6],
                    in_=ybuf[:, 12:14, :].rearrange("p k d -> p (k d)"),
                )
            elif st == 15:
                nc.sync.dma_start(
                    out=yh_view[:, 7],
                    in_=ybuf[:, 14:16, :].rearrange("p k d -> p (k d)"),
                )
            elif st % 4 == 3:
                c = st // 4
                nc.sync.dma_start(
                    out=y_view[:, c],
                    in_=ybuf[:, 4 * c : 4 * c + 4, :].rearrange("p k d -> p (k d)"),
                )

        # flat software pipeline over all (j, i) steps: scores/exp run LAG
        # steps ahead of AV, crossing chunk boundaries so neither PE nor ACT
        # drains at chunk turns.  Norms are delayed NDELAY further steps so
        # their PE broadcast matmuls never gate the scores stream, and each
        # chunk's out-projection is queued behind the following chunk's AV
        # (it steals "av" PSUM slots, so it runs once those free up).
        LAG = int(os.environ.get("LAG", "4"))
        NDELAY = int(os.environ.get("NDELAY", "0"))
        # out-projections drain into the following chunks' step stream (one
        # stile every OP_EVERY steps) so y DMA streams throughout the kernel
        # instead of piling into a tail; 0 disables (all at the end).
        OP_EARLY = int(os.environ.get("OP_EARLY", "2"))
        OP_FROM = int(os.environ.get("OP_FROM", "3"))  # first chunk that interleaves oproj
        steps = [(j, i) for j in range(QC) for i in range(4 * (j + 1))]
        av_of: dict = {}
        es_of: dict = {}
        es_pair: dict = {}
        gate: list = []
        work_q: list = []  # deferred (fn, args) emissions
        pend_oproj: list = []

        def do_av(idx):
            pj, pi = steps[idx]
            nkp = 4 * (pj + 1)
            if pi == 0:
                av_of[pj] = [
                    ps_av.tile([128, 512], F32, tag="av", name=f"av_{pj}_{h}")
                    for h in range(HPC)
                ]
            emit_av(pj, pi, nkp, es_of.pop((pj, pi)), av_of[pj])
            if pi == nkp - 1:
                work_q.append(("norm", pj, NDELAY))

        def drain_work_q():
            rest = []
            for kind, arg, delay in work_q:
                if delay > 0:
                    rest.append((kind, arg, delay - 1))
                    continue
                if kind == "norm":
                    emit_norm(arg, av_of.pop(arg))
                    if OP_EARLY:
                        pend_oproj.extend(range(4 * arg, 4 * arg + 4))
            work_q[:] = rest

        # proj chunk c+1's pieces are spread across attention chunk c's steps
        # (attention chunk j only needs projection chunks <= j).
        # PE pipeline refills (~160ns) are paid at every matmul SHAPE switch
        # (scores (64,128) / AV (128,65) / proj+oproj (128,128)), so emission
        # batches same-shape work: AVs in PAIR-step bursts, proj pieces at
        # 4-step boundaries.
        PAIR = int(os.environ.get("PAIR", "2"))
        AB_DEFER = int(os.environ.get("AB_DEFER", "0"))
        OP_PHASE = int(os.environ.get("OP_PHASE", "0"))
        TAILALT = int(os.environ.get("TAILALT", "12"))
        pend_ab: list = []
        for piece in proj_pieces(0):
            piece()
        pend_proj: list = list(proj_pieces(1))
        for idx, (j, i) in enumerate(steps):
            nk = 4 * (j + 1)
            if pend_proj:
                quota = max(1, -(-len(pend_proj) // max(1, nk - i)))
                for _ in range(quota):
                    if pend_proj:
                        pend_proj.pop(0)()
            if i == nk - 1 and j + 2 < QC:
                pend_proj = list(proj_pieces(j + 2))
            es_of[(j, i)], ab_fn = emit_scores(j, i)
            # defer each step's AB exp by AB_DEFER steps on the ACT stream:
            # the single-buffered C path gains a step of slack (its expC no
            # longer queues behind the previous step's big AB exp)
            pend_ab.append(ab_fn)
            while len(pend_ab) > AB_DEFER:
                pend_ab.pop(0)()
            if idx >= LAG and (idx - LAG) % PAIR == PAIR - 1:
                for k in range(PAIR):
                    do_av(idx - LAG - PAIR + 1 + k)
            drain_work_q()
            if OP_EARLY and pend_oproj and j >= OP_FROM and i % OP_EARLY == OP_PHASE:
                emit_oproj(pend_oproj.pop(0))
        while pend_ab:
            pend_ab.pop(0)()
        done_av = len(steps) - LAG
        done_av -= (done_av % PAIR)
        for idx in range(done_av, len(steps)):
            do_av(idx)
            drain_work_q()
        while work_q:
            drain_work_q()
        for st in (list(pend_oproj) if OP_EARLY else range(ST)):
            emit_oproj(st)


def _build():
    if "nc" in _CACHE:
        return _CACHE["nc"]
    nc = bacc.Bacc("TRN2", target_bir_lowering=False, debug=False, num_devices=NCORE)
    dr = {
        "xT": nc.dram_tensor("xT", [128, FT * S], BF16, kind="ExternalInput"),
        "wqk": nc.dram_tensor("wqk", [128, FT * 3 * 128], BF16, kind="ExternalInput"),
        "wv": nc.dram_tensor("wv", [128, FT * 192], BF16, kind="ExternalInput"),
        "woAB": nc.dram_tensor("woAB", [128, D], BF16, kind="ExternalInput"),
        "woC": nc.dram_tensor("woC", [128, D], BF16, kind="ExternalInput"),
        "bqk": nc.dram_tensor("bqk", [128, 3], F32, kind="ExternalInput"),
        "bv": nc.dram_tensor("bv", [128, 192], F32, kind="ExternalInput"),
        "mask": nc.dram_tensor("mask", [128, 128], BF16, kind="ExternalInput"),
    }
    y_dr = nc.dram_tensor("y", [128, ST * D], mybir.dt.float16, kind="ExternalOutput")
    with tile.TileContext(nc) as tc:
        _emit(nc, tc, dr, y_dr)
    nc.compile()
    _CACHE["nc"] = nc
    return nc


def prep_inputs(x, Wq, bq, Wk, bk, Wv, bv, Wo, bo):
    """Shard + pre-layout the full fp32 inputs into 8 per-core input maps."""
    in_maps = []
    mask = (np.arange(128)[:, None] <= np.arange(128)[None, :]).astype(NPBF)
    for c in range(NCORE):
        b, g = c // 4, c % 4
        hs = [3 * g, 3 * g + 1, 3 * g + 2]

        xT = np.ascontiguousarray(
            x[b].T.reshape(FT, 128, QC, 512).transpose(1, 2, 0, 3)
        )  # [128, QC, FT, 512] chunk-major

        def rows(W, h):
            return W[h * 64 : (h + 1) * 64]  # [64, D]

        G0 = np.concatenate([rows(Wq, hs[0]), rows(Wq, hs[1])], 0)  # [128, D]
        G1 = np.concatenate([rows(Wk, hs[0]), rows(Wk, hs[1])], 0)
        G2 = np.concatenate([rows(Wq, hs[2]), rows(Wk, hs[2])], 0)
        # wqk[p, g, f, m] = G_g[m, f*128+p]  (g-major so group 0 streams first)
        wqk = np.stack([G0, G1, G2], 0).transpose(2, 0, 1)  # [D, 3, 128]
        wqk = wqk.reshape(FT, 128, 3, 128).transpose(1, 2, 0, 3)  # [128, 3, FT, 128]

        Vg = Wv[g * 192 : (g + 1) * 192]  # [192, D]
        wv_ = Vg.T.reshape(FT, 128, 192).transpose(1, 0, 2)  # [128, FT, 192]

        # out-proj rhs: rows = local head dims, cols = output features
        woAB = np.concatenate(
            [
                Wo[:, (3 * g + 0) * 64 : (3 * g + 1) * 64].T,
                Wo[:, (3 * g + 1) * 64 : (3 * g + 2) * 64].T,
            ],
            0,
        )  # [128, D]
        woC = np.zeros((128, D), np.float32)
        woC[0:64] = Wo[:, (3 * g + 2) * 64 : (3 * g + 3) * 64].T
        woC[64:128] = woC[0:64]  # duplicated so odd stiles read rows 64-127

        bqk_ = np.stack(
            [
                np.concatenate([bq[hs[0] * 64 : hs[0] * 64 + 64], bq[hs[1] * 64 : hs[1] * 64 + 64]]),
                np.concatenate([bk[hs[0] * 64 : hs[0] * 64 + 64], bk[hs[1] * 64 : hs[1] * 64 + 64]]),
                np.concatenate([bq[hs[2] * 64 : hs[2] * 64 + 64], bk[hs[2] * 64 : hs[2] * 64 + 64]]),
            ],
            1,
        ).astype(np.float32)  # [128, 3]

        bv_ = np.tile(bv[g * 192 : (g + 1) * 192][None, :], (128, 1)).astype(np.float32)

        in_maps.append(
            {
                "xT": xT.reshape(128, FT * S).astype(NPBF),
                "wqk": wqk.reshape(128, FT * 3 * 128).astype(NPBF),
                "wv": wv_.reshape(128, FT * 192).astype(NPBF),
                "woAB": woAB.astype(NPBF),
                "woC": woC.astype(NPBF),
                "bqk": bqk_,
                "bv": bv_,
                "mask": mask,
            }
        )
    return in_maps


def run_spmd(in_maps, trace=False, **kw):
    nc = _build()
    return run_bass_kernel_spmd(nc, in_maps, core_ids=list(range(NCORE)), trace=trace, **kw)


def gather(results, bo=None):
    y = np.zeros((B, S, D), np.float32)
    for c in range(NCORE):
        # y dram is [128, ST, D] partition-major: y_full[st*128+p] = y[p, st]
        yp = np.asarray(results[c]["y"], np.float32).reshape(128, ST, D)
        y[c // 4] += yp.transpose(1, 0, 2).reshape(S, D)
    if bo is not None:
        y += np.asarray(bo, np.float32)
    return y


def kernel(x, Wq, bq, Wk, bk, Wv, bv, Wo, bo):
    args = [np.asarray(a, np.float32) for a in (x, Wq, bq, Wk, bk, Wv, bv, Wo, bo)]
    in_maps = prep_inputs(*args)
    last_err = None
    for _attempt in range(3):
        try:
            res = run_spmd(in_maps)
            return gather(res.results, bo=args[8])
        except Exception as e:  # transient NRT/axon hiccups: retry
            last_err = e
            import time

            time.sleep(2.0)
    raise last_err



# revision 33
# speedup vs baseline: 1.2441x; 1.0252x over previous
"""Causal multi-head attention (B=2, S=2048, D=768, H=12) on 8 Trainium2 cores.

Sharding: core c -> batch b = c//4, head-group g = c%4 (heads 3g..3g+2).
Each core computes its 3 heads end-to-end in bf16 (fp32 PSUM accumulation)
and produces a partial output-projection y_partial[b] = out_g @ Wo_g^T
(+ bo on g==0 cores).  The host sums the 4 partials per batch (the
"all-reduce") while unsharding.

Device layout notes (per core):
  qkT groups (dim-on-partition, token-on-free), each [128, 2048] bf16:
    G0 = [q_h0 (p0-63) ; q_h1 (p64-127)]
    G1 = [k_h0 ; k_h1]
    G2 = [q_h2 ; k_h2]
  q_h2 is DMA-rehomed to partitions 64-127 and k_h2 to partitions 0-63 so
  head-2 score matmuls can alternate between PE row-groups (load balance
  against heads 0/1 which are pinned to row groups 0 and 64).
  Scores are computed transposed S_T[k, q] so the softmax denominator
  falls out of the AV matmul via a ones column appended to v.  The
  denominator row is broadcast across partitions with a tiny ones-matmul,
  reciprocal'd on the DVE, and multiplied into the AV rows.
"""

import os
import sys

import numpy as np

for _p in ("/opt/trn_rl_repo",):
    if _p not in sys.path and os.path.isdir(_p):
        sys.path.insert(0, _p)

import ml_dtypes  # noqa: E402

import concourse.bass as bass  # noqa: E402
import concourse.mybir as mybir  # noqa: E402
import concourse.tile as tile  # noqa: E402
from concourse import bacc  # noqa: E402
from concourse.bass_utils import run_bass_kernel_spmd  # noqa: E402
from concourse.tile_rust import add_dep_helper  # noqa: E402

BF16 = mybir.dt.bfloat16
F32 = mybir.dt.float32
F8 = mybir.dt.float8e4
NPBF = ml_dtypes.bfloat16
NPF8 = ml_dtypes.float8_e4m3
WSCALE = 16.0  # host premultiplier on Wq/Wk so fp8 stays out of subnormals
ESHIFT = float(-np.log(16.0))  # exp bias: es scaled by 1/16 to fit fp8e4 range
DR = mybir.MatmulPerfMode.DoubleRow

B, S, D = 2, 2048, 768
H, HD = 12, 64
NCORE = 8
HPC = 3  # heads per core
FT = D // 128  # 6 contraction tiles for projections
ST = S // 128  # 16 token tiles
QC = S // 512  # 4 q-chunks of 512
SCALE = float(1.0 / np.sqrt(HD))

_CACHE: dict = {}


def _emit(nc: bacc.Bacc, tc: tile.TileContext, dr: dict, y_dr) -> None:
    from contextlib import ExitStack

    Exp = mybir.ActivationFunctionType.Exp

    with ExitStack() as ex:
        pool = lambda name, bufs, space="SBUF": ex.enter_context(  # noqa: E731
            tc.tile_pool(name=name, bufs=bufs, space=space)
        )

        consts = pool("consts", 1)

        # ---- persistent SBUF tensors -------------------------------------
        xT = consts.tile([128, QC, FT, 512], BF16)  # x[b]^T, chunk-major
        wqk = consts.tile([128, 3, FT, 128], BF16)  # qk projection lhsT tiles (g-major)
        wv = consts.tile([128, FT, 192], BF16)  # v projection rhs tiles
        woAB = consts.tile([128, D], BF16)  # out-proj rhs, heads 0+1 packed
        woC = consts.tile([128, D], BF16)  # out-proj rhs, head 2 (rows duplicated)
        bqk = consts.tile([128, 3], F32)
        bv = consts.tile([128, 192], F32)
        mask = consts.tile([128, 128], BF16)  # tri mask m[p,c]=1 if p<=c
        ones = consts.tile([65, 64], mybir.dt.float16)  # bcast matmul lhsT (row 64)
        ebias = consts.tile([128, 1], F32)  # exp bias (-ln16) per partition

        qkT = consts.tile([128, 3, S], BF16)  # projected q/k groups (16x scaled)
        qCmv = consts.tile([128, S], BF16)  # q_h2 rehomed to partitions 64-127
        kCmv = consts.tile([128, S], BF16)  # k_h2 rehomed to partitions 0-63
        vsb = consts.tile([128, ST, HPC, 65], BF16)  # v (+ones col) per ktile
        vsb8 = consts.tile([128, 12, HPC, 80], F8)  # fp8 v (+ones col), ktiles 0-11
        outAB = consts.tile([128, S], BF16)  # normalized out_T heads 0 (+1 moved)
        outC = consts.tile([128, S], BF16)  # normalized out_T head 2; even stiles
        # live on partitions 0-63, odd stiles on 64-127 so tail C-oprojs pair
        outB = consts.tile([64, S], BF16)  # normalized out_T head 1 (pre-move)

        # Input streaming.  Measured queue behavior: one queue's entries
        # pipeline at ~240GB/s; multiple queues share ~350-400GB/s of fabric.
        # The first projection chain needs wqk + x chunk 0, so that set is
        # split ACROSS queues (sync: wqk+x0a; gpsimd: x0b) to use the full
        # fabric, and the only early bulk competitor (scalar queue: woAB+xc2)
        # is issue-gated behind x0's completion.
        # The first projection chain needs wqk[g0] + x chunk 0; those three
        # pieces are spread across the three DMA queues so they land in
        # parallel right after the NEFF preamble.  Queue-sequential ordering
        # then prioritizes the rest by first use.
        xview = dr["xT"].ap().rearrange("p (c f s) -> p c f s", c=QC, f=FT)
        wqkv = dr["wqk"].ap().rearrange("p (g f m) -> p g f m", g=3, f=FT)
        nc.sync.dma_start(out=wqk[:, 0], in_=wqkv[:, 0])
        nc.sync.dma_start(out=wqk[:, 1:3], in_=wqkv[:, 1:3])
        nc.sync.dma_start(out=xT[:, 1, :, :], in_=xview[:, 1, :, :])
        nc.sync.dma_start(out=xT[:, 3, :, :], in_=xview[:, 3, :, :])
        nc.gpsimd.dma_start(out=xT[:, 0, 0:3, :], in_=xview[:, 0, 0:3, :])
        nc.gpsimd.dma_start(
            out=wv[:], in_=dr["wv"].ap().rearrange("p (f m) -> p f m", f=FT)
        )
        nc.gpsimd.dma_start(out=woC[:], in_=dr["woC"].ap())
        nc.scalar.dma_start(out=xT[:, 0, 3:FT, :], in_=xview[:, 0, 3:FT, :])
        nc.scalar.dma_start(out=bqk[:], in_=dr["bqk"].ap())
        nc.scalar.dma_start(out=mask[:], in_=dr["mask"].ap())
        nc.scalar.dma_start(out=bv[:], in_=dr["bv"].ap())
        nc.scalar.dma_start(out=woAB[:], in_=dr["woAB"].ap())
        nc.scalar.dma_start(out=xT[:, 2, :, :], in_=xview[:, 2, :, :])
        nc.vector.memset(vsb[:, :, :, 64:65], 1.0)
        nc.vector.memset(vsb8[:, :, :, 64:65], 1.0)
        nc.vector.memset(ones[64:65, :], 1.0)
        nc.vector.memset(ebias[:], ESHIFT)

        # ---- PSUM pools (8 banks total, statically allocated) -------------
        # ps_big: 2 slots x 2 banks  -> qk-proj, v-proj, scores(A,B), out-proj
        # ps_av : 3 slots x 1 bank   -> AV accumulators
        # ps_sm : 1 slot  x 1 bank   -> scores(C)
        ps_big = pool("ps_big", 2, "PSUM")
        ps_av = pool("ps_av", 3, "PSUM")
        ps_sm = pool("ps_sm", 1, "PSUM")

        # ---- projection pieces (emitted interleaved with attention) --------
        def emit_proj_qk(q4, g):
            qs = slice(512 * q4, 512 * (q4 + 1))
            ps = ps_big.tile([128, 1024], F32, tag="big", name=f"qkp_{g}_{q4}")
            for f in range(FT):
                nc.tensor.matmul(
                    ps[:, 0:512],
                    lhsT=wqk[:, g, f, :],
                    rhs=xT[:, q4, f, :],
                    start=(f == 0),
                    stop=(f == FT - 1),
                )
            nc.vector.tensor_scalar_add(qkT[:, g, qs], ps[:, 0:512], bqk[:, g : g + 1])
            if g == 2:
                # rehome head-2 q/k so C-scores can run on either row-group
                nc.sync.dma_start(out=qCmv[64:128, qs], in_=qkT[0:64, 2, qs])
                nc.sync.dma_start(out=kCmv[0:64, qs], in_=qkT[64:128, 2, qs])

        def emit_proj_v(st):
            ps = ps_big.tile([128, 1024], F32, tag="big", name=f"vp_{st}")
            c, sub = st // 4, st % 4
            for f in range(FT):
                nc.tensor.matmul(
                    ps[:, 0:192],
                    lhsT=xT[:, c, f, 128 * sub : 128 * (sub + 1)],
                    rhs=wv[:, f, :],
                    start=(f == 0),
                    stop=(f == FT - 1),
                )
            nc.vector.tensor_add(
                vsb[:, st, :, 0:64],
                ps[:, 0:192].rearrange("p (h d) -> p h d", h=3),
                bv[:].rearrange("p (h d) -> p h d", h=3),
            )
            if st < 12:  # fp8 copy for DoubleRow AV (off-diagonal ktiles only)
                nc.vector.tensor_copy(vsb8[:, st, :, 0:64], vsb[:, st, :, 0:64])

        def proj_pieces(c):
            out = [lambda g=g: emit_proj_qk(c, g) for g in range(3)]
            out += [lambda st=st: emit_proj_v(st) for st in range(4 * c, 4 * c + 4)]
            return out

        # ---- attention -----------------------------------------------------
        # es for off-diagonal ("full") steps lives in fp8 PAIR tiles (two
        # consecutive ktiles) so the AV can run as fp8 DoubleRow matmuls
        # contracting 256 k at once.  Diagonal steps stay bf16 (their queries
        # include short-prefix tokens where attention concentrates and fp8 v
        # noise would not average out).  ALL exps carry bias=-ln16 so fp8 es
        # stays in range; the 1/16 cancels in the softmax normalization.
        exp_sb = pool("exp_sb", int(os.environ.get("ESB", "10")))
        exp8_sb = pool("exp8_sb", int(os.environ.get("ESB8", "6")))
        den_sb = pool("den_sb", 3)
        rec_sb = pool("rec_sb", 3)
        SCL = SCALE

        NODR = os.environ.get("NODR", "0") == "1"

        def emit_scores(j, i):
            full = i < 4 * j and not NODR
            off = 0 if full else 128 * (i - 4 * j)
            qs = slice(512 * j + off, 512 * (j + 1))
            ks = slice(128 * i, 128 * (i + 1))
            sAB_raw = ps_big.tile([128, 1024], F32, tag="big", name=f"sAB_{j}_{i}")
            sAB = sAB_raw[:].rearrange("p (h q) -> p h q", h=2)
            sC = ps_sm.tile([128, 512], F32, tag="sm", name=f"sC_{j}_{i}")
            mmA = nc.tensor.matmul(
                sAB[:, 0, off:], lhsT=qkT[0:64, 1, ks], rhs=qkT[0:64, 0, qs]
            )
            if i == 0:
                gate.clear()
            if i < 2:
                gate.append(mmA)
            nc.tensor.matmul(
                sAB[:, 1, off:], lhsT=qkT[64:128, 1, ks], rhs=qkT[64:128, 0, qs]
            )
            if i % 2 == 0:
                nc.tensor.matmul(sC[:, off:], lhsT=kCmv[0:64, ks], rhs=qkT[0:64, 2, qs])
            else:
                nc.tensor.matmul(
                    sC[:, off:], lhsT=qkT[64:128, 2, ks], rhs=qCmv[64:128, qs]
                )
            if full:
                if i % 2 == 0:
                    es = exp8_sb.tile(
                        [128, 2, HPC, 512], F8, tag="es8", name=f"es8_{j}_{i}"
                    )
                    es_pair[(j, i + 1)] = es
                else:
                    es = es_pair.pop((j, i))
                sl = i % 2
                nc.scalar.activation(
                    es[:, sl, 2, :], sC[:, 0:512], Exp, scale=SCL, bias=ebias[:]
                )

                def emit_ab():
                    nc.scalar.activation(
                        es[:, sl, 0:2, :].rearrange("p h q -> p (h q)"),
                        sAB_raw[:, 0:1024],
                        Exp,
                        scale=SCL,
                        bias=ebias[:],
                    )

                return ("f8", es, sl), emit_ab
            es = exp_sb.tile([128, HPC, 512], BF16, tag="es", name=f"es_{j}_{i}")
            # exp C first: sC is single-buffered, so freeing it early keeps
            # the next step's C matmul off the ACT critical path (sAB has 2
            # slots and tolerates the extra lag)
            nc.scalar.activation(es[:, 2, off:], sC[:, off:], Exp, scale=SCL, bias=ebias[:])

            def emit_ab():
                if off == 0:
                    # contiguous fast path: flat 1-D APs for the A|B pair
                    nc.scalar.activation(
                        es[:].rearrange("p h q -> p (h q)")[:, 0:1024],
                        sAB_raw[:, 0:1024],
                        Exp,
                        scale=SCL,
                        bias=ebias[:],
                    )
                else:
                    nc.scalar.activation(
                        es[:, 0:2, off:], sAB[:, :, off:], Exp, scale=SCL, bias=ebias[:]
                    )

            return ("bf", es, off), emit_ab

        def emit_av(j, i, nk, esrec, av):
            kind, es, aux = esrec
            if kind == "f8":
                if i % 2 == 0:
                    return  # handled together with the odd pair member
                p = i - 1
                for h in range(HPC):
                    nc.tensor.matmul(
                        av[h][0:65, :],
                        lhsT=vsb8[:, p : p + 2, h, 0:65],
                        rhs=es[:, :, h, :],
                        start=(p == 0),
                        stop=False,
                        perf_mode=DR,
                    )
                return
            off = aux
            if i >= 4 * j:  # diagonal block: zero the k>q half
                dm = slice(off, off + 128)
                nc.vector.tensor_mul(
                    es[:, :, dm],
                    es[:, :, dm],
                    mask[:, None, :].broadcast_to([128, HPC, 128]),
                )
            for h in range(HPC):
                mm = nc.tensor.matmul(
                    av[h][0:65, off:],
                    lhsT=vsb[:, i, h, :],
                    rhs=es[:, h, off:],
                    start=(i == 0),
                    stop=(i == nk - 1),
                )
                # at the last chunk boundary, let its first scores beat the
                # prior chunk's AV backlog onto the PE stream (ordering only)
                if i >= nk - 3 and gate:
                    for gmm in gate:
                        add_dep_helper(
                            mm.ins, gmm.ins, sync=False,
                            reason="boundary: scores before AV backlog",
                        )

        def emit_norm(j, av):
            qs_full = slice(512 * j, 512 * (j + 1))
            # normalization: out = outU * (1/denom) ; denom = av row 64.
            # Stage-ordered (copies, broadcasts, recips, muls) so the first
            # AV slot frees as early as possible.
            dens = []
            for h in range(HPC):
                den = den_sb.tile(
                    [65, 512], mybir.dt.float16, tag="den", name=f"dn_{j}_{h}"
                )
                nc.vector.tensor_copy(den[64:65, 0:512], av[h][64:65, :])
                dens.append(den)
            # h2's broadcast keeps the single-slot sC pool (allocated first so
            # it frees fastest); h1/h0 share halves of one big-pool tile so
            # the sC pool isn't serialized behind the whole norm.  h1 before
            # h0: its result needs a rehome DMA before the packed out-proj.
            bigbc = ps_big.tile([128, 1024], F32, tag="big", name=f"bch_{j}")
            bc_ap = {
                2: ps_sm.tile([128, 512], F32, tag="sm", name=f"b_{j}_2"),
                1: bigbc[:, 0:512],
                0: bigbc[:, 512:1024],
            }
            for h in (2, 1, 0):
                bc = bc_ap[h]
                nc.tensor.matmul(
                    bc[0:64, :], lhsT=ones[64:65, :], rhs=dens[h][64:65, 0:512]
                )
                rec = rec_sb.tile([64, 512], F32, tag="rec", name=f"rc_{j}_{h}")
                nc.vector.reciprocal_approx_fast(rec[:], bc[0:64, :])
                dst = (outAB[0:64, qs_full], outB[:, qs_full], outC[0:64, qs_full])[h]
                nc.vector.tensor_mul(dst, av[h][0:64, :], rec[:])
                if h == 1:
                    # move head-1 slice onto partitions 64-127 for out-proj
                    nc.sync.dma_start(out=outAB[64:128, qs_full], in_=outB[:, qs_full])
                if h == 2:
                    # rehome ODD stiles' h2 rows to partitions 64-127 so the
                    # tail's C-oprojs can pair across PE row groups
                    odd = outC[0:64, qs_full].rearrange("p (a c) -> p a c", a=4)
                    oddhi = outC[64:128, qs_full].rearrange("p (a c) -> p a c", a=4)
                    nc.sync.dma_start(out=oddhi[:, 1], in_=odd[:, 1])
                    nc.sync.dma_start(out=oddhi[:, 3], in_=odd[:, 3])

        # y stages in a persistent SBUF buffer (partition-major DRAM layout so
        # per-chunk DMAs move 6KB runs; host re-lays out to [S, D])
        ybuf = consts.tile([128, ST, D], mybir.dt.float16)
        y_view = y_dr.ap().rearrange("p (c k d) -> p c (k d)", c=QC, k=4)
        yh_view = y_dr.ap().rearrange("p (hh d) -> p hh d", hh=8)

        def emit_oproj_ab(st):
            ss = slice(128 * st, 128 * (st + 1))
            ps = ps_big.tile([128, 1024], F32, tag="big", name=f"yp_{st}")
            for n0, nw in ((0, 512), (512, 256)):
                nc.tensor.matmul(
                    ps[:, n0 : n0 + nw],
                    lhsT=outAB[:, ss],
                    rhs=woAB[:, n0 : n0 + nw],
                    start=True,
                    stop=False,
                )
            return ps

        def emit_oproj_c(st, ps):
            ss = slice(128 * st, 128 * (st + 1))
            rows = slice(64 * (st % 2), 64 * (st % 2) + 64)
            for n0, nw in ((0, 512), (512, 256)):
                nc.tensor.matmul(
                    ps[:, n0 : n0 + nw],
                    lhsT=outC[rows, ss],
                    rhs=woC[rows, n0 : n0 + nw],
                    start=False,
                    stop=True,
                )

        def emit_oproj(st):
            # both AB matmuls first, then both C: the stationary weights
            # change once per stile instead of three times (each PSUM region
            # still gets its own start->stop accumulation pair)
            ps = emit_oproj_ab(st)
            emit_oproj_c(st, ps)
            emit_oproj_fin(st, ps)

        def emit_oproj_fin(st, ps):
            # fp16 staging copy: DVE while ACT is busy with exp; in the last
            # chunk (exp winding down) alternate onto ACT so the tail's copies
            # don't serialize on one engine
            # stiles 8-15 run post-exp (OP_FROM=3 pops only 0-7 mid-chunk-3),
            # where ACT is idle and 8 serialized DVE copies (~7.6us) would be
            # the tail's critical path — alternate DVE/ACT there
            if st >= TAILALT and st % 2 == 1:
                nc.scalar.copy(ybuf[:, st, :], ps[:, 0:D])
            else:
                nc.vector.tensor_copy(ybuf[:, st, :], ps[:, 0:D])
            if st == 13:
                # last chunk: split across both HWDGE queues so the final y
                # bytes overlap the remaining out-proj matmuls (safe here:
                # ACT's exp stream is done, so its issue-wait can't stall exp)
                nc.scalar.dma_start(
                    out=yh_view[:, 6],
                    in_=ybuf[:, 12:14, :].rearrange("p k d -> p (k d)"),
                )
            elif st == 15:
                nc.sync.dma_start(
                    out=yh_view[:, 7],
                    in_=ybuf[:, 14:16, :].rearrange("p k d -> p (k d)"),
                )
            elif st % 4 == 3:
                c = st // 4
                nc.sync.dma_start(
                    out=y_view[:, c],
                    in_=ybuf[:, 4 * c : 4 * c + 4, :].rearrange("p k d -> p (k d)"),
                )

        # flat software pipeline over all (j, i) steps: scores/exp run LAG
        # steps ahead of AV, crossing chunk boundaries so neither PE nor ACT
        # drains at chunk turns.  Norms are delayed NDELAY further steps so
        # their PE broadcast matmuls never gate the scores stream, and each
        # chunk's out-projection is queued behind the following chunk's AV
        # (it steals "av" PSUM slots, so it runs once those free up).
        LAG = int(os.environ.get("LAG", "4"))
        NDELAY = int(os.environ.get("NDELAY", "0"))
        # out-projections drain into the following chunks' step stream (one
        # stile every OP_EVERY steps) so y DMA streams throughout the kernel
        # instead of piling into a tail; 0 disables (all at the end).
        OP_EARLY = int(os.environ.get("OP_EARLY", "2"))
        OP_FROM = int(os.environ.get("OP_FROM", "3"))  # first chunk that interleaves oproj
        steps = [(j, i) for j in range(QC) for i in range(4 * (j + 1))]
        av_of: dict = {}
        es_of: dict = {}
        es_pair: dict = {}
        gate: list = []
        work_q: list = []  # deferred (fn, args) emissions
        pend_oproj: list = []

        def do_av(idx):
            pj, pi = steps[idx]
            nkp = 4 * (pj + 1)
            if pi == 0:
                av_of[pj] = [
                    ps_av.tile([128, 512], F32, tag="av", name=f"av_{pj}_{h}")
                    for h in range(HPC)
                ]
            emit_av(pj, pi, nkp, es_of.pop((pj, pi)), av_of[pj])
            if pi == nkp - 1:
                work_q.append(("norm", pj, NDELAY))

        def drain_work_q():
            rest = []
            for kind, arg, delay in work_q:
                if delay > 0:
                    rest.append((kind, arg, delay - 1))
                    continue
                if kind == "norm":
                    emit_norm(arg, av_of.pop(arg))
                    if OP_EARLY:
                        pend_oproj.extend(range(4 * arg, 4 * arg + 4))
            work_q[:] = rest

        # proj chunk c+1's pieces are spread across attention chunk c's steps
        # (attention chunk j only needs projection chunks <= j).
        # PE pipeline refills (~160ns) are paid at every matmul SHAPE switch
        # (scores (64,128) / AV (128,65) / proj+oproj (128,128)), so emission
        # batches same-shape work: AVs in PAIR-step bursts, proj pieces at
        # 4-step boundaries.
        PAIR = int(os.environ.get("PAIR", "2"))
        AB_DEFER = int(os.environ.get("AB_DEFER", "0"))
        OP_PHASE = int(os.environ.get("OP_PHASE", "0"))
        TAILALT = int(os.environ.get("TAILALT", "12"))
        pend_ab: list = []
        for piece in proj_pieces(0):
            piece()
        pend_proj: list = list(proj_pieces(1))
        for idx, (j, i) in enumerate(steps):
            nk = 4 * (j + 1)
            if pend_proj:
                quota = max(1, -(-len(pend_proj) // max(1, nk - i)))
                for _ in range(quota):
                    if pend_proj:
                        pend_proj.pop(0)()
            if i == nk - 1 and j + 2 < QC:
                pend_proj = list(proj_pieces(j + 2))
            es_of[(j, i)], ab_fn = emit_scores(j, i)
            # defer each step's AB exp by AB_DEFER steps on the ACT stream:
            # the single-buffered C path gains a step of slack (its expC no
            # longer queues behind the previous step's big AB exp)
            pend_ab.append(ab_fn)
            while len(pend_ab) > AB_DEFER:
                pend_ab.pop(0)()
            if idx >= LAG and (idx - LAG) % PAIR == PAIR - 1:
                for k in range(PAIR):
                    do_av(idx - LAG - PAIR + 1 + k)
            drain_work_q()
            if OP_EARLY and pend_oproj and j >= OP_FROM and i % OP_EARLY == OP_PHASE:
                emit_oproj(pend_oproj.pop(0))
        while pend_ab:
            pend_ab.pop(0)()
        done_av = len(steps) - LAG
        done_av -= (done_av % PAIR)
        for idx in range(done_av, len(steps)):
            do_av(idx)
            drain_work_q()
        while work_q:
            drain_work_q()
        for st in (list(pend_oproj) if OP_EARLY else range(ST)):
            emit_oproj(st)


def _build():
    if "nc" in _CACHE:
        return _CACHE["nc"]
    nc = bacc.Bacc("TRN2", target_bir_lowering=False, debug=False, num_devices=NCORE)
    dr = {
        "xT": nc.dram_tensor("xT", [128, FT * S], BF16, kind="ExternalInput"),
        "wqk": nc.dram_tensor("wqk", [128, FT * 3 * 128], BF16, kind="ExternalInput"),
        "wv": nc.dram_tensor("wv", [128, FT * 192], BF16, kind="ExternalInput"),
        "woAB": nc.dram_tensor("woAB", [128, D], BF16, kind="ExternalInput"),
        "woC": nc.dram_tensor("woC", [128, D], BF16, kind="ExternalInput"),
        "bqk": nc.dram_tensor("bqk", [128, 3], F32, kind="ExternalInput"),
        "bv": nc.dram_tensor("bv", [128, 192], F32, kind="ExternalInput"),
        "mask": nc.dram_tensor("mask", [128, 128], BF16, kind="ExternalInput"),
    }
    y_dr = nc.dram_tensor("y", [128, ST * D], mybir.dt.float16, kind="ExternalOutput")
    with tile.TileContext(nc) as tc:
        _emit(nc, tc, dr, y_dr)
    nc.compile()
    _CACHE["nc"] = nc
    return nc


def prep_inputs(x, Wq, bq, Wk, bk, Wv, bv, Wo, bo):
    """Shard + pre-layout the full fp32 inputs into 8 per-core input maps."""
    in_maps = []
    mask = (np.arange(128)[:, None] <= np.arange(128)[None, :]).astype(NPBF)
    for c in range(NCORE):
        b, g = c // 4, c % 4
        hs = [3 * g, 3 * g + 1, 3 * g + 2]

        xT = np.ascontiguousarray(
            x[b].T.reshape(FT, 128, QC, 512).transpose(1, 2, 0, 3)
        )  # [128, QC, FT, 512] chunk-major

        def rows(W, h):
            return W[h * 64 : (h + 1) * 64]  # [64, D]

        G0 = np.concatenate([rows(Wq, hs[0]), rows(Wq, hs[1])], 0)  # [128, D]
        G1 = np.concatenate([rows(Wk, hs[0]), rows(Wk, hs[1])], 0)
        G2 = np.concatenate([rows(Wq, hs[2]), rows(Wk, hs[2])], 0)
        # wqk[p, g, f, m] = G_g[m, f*128+p]  (g-major so group 0 streams first)
        wqk = np.stack([G0, G1, G2], 0).transpose(2, 0, 1)  # [D, 3, 128]
        wqk = wqk.reshape(FT, 128, 3, 128).transpose(1, 2, 0, 3)  # [128, 3, FT, 128]

        Vg = Wv[g * 192 : (g + 1) * 192]  # [192, D]
        wv_ = Vg.T.reshape(FT, 128, 192).transpose(1, 0, 2)  # [128, FT, 192]

        # out-proj rhs: rows = local head dims, cols = output features
        woAB = np.concatenate(
            [
                Wo[:, (3 * g + 0) * 64 : (3 * g + 1) * 64].T,
                Wo[:, (3 * g + 1) * 64 : (3 * g + 2) * 64].T,
            ],
            0,
        )  # [128, D]
        woC = np.zeros((128, D), np.float32)
        woC[0:64] = Wo[:, (3 * g + 2) * 64 : (3 * g + 3) * 64].T
        woC[64:128] = woC[0:64]  # duplicated so odd stiles read rows 64-127

        bqk_ = np.stack(
            [
                np.concatenate([bq[hs[0] * 64 : hs[0] * 64 + 64], bq[hs[1] * 64 : hs[1] * 64 + 64]]),
                np.concatenate([bk[hs[0] * 64 : hs[0] * 64 + 64], bk[hs[1] * 64 : hs[1] * 64 + 64]]),
                np.concatenate([bq[hs[2] * 64 : hs[2] * 64 + 64], bk[hs[2] * 64 : hs[2] * 64 + 64]]),
            ],
            1,
        ).astype(np.float32)  # [128, 3]

        bv_ = np.tile(bv[g * 192 : (g + 1) * 192][None, :], (128, 1)).astype(np.float32)

        in_maps.append(
            {
                "xT": xT.reshape(128, FT * S).astype(NPBF),
                "wqk": wqk.reshape(128, FT * 3 * 128).astype(NPBF),
                "wv": wv_.reshape(128, FT * 192).astype(NPBF),
                "woAB": woAB.astype(NPBF),
                "woC": woC.astype(NPBF),
                "bqk": bqk_,
                "bv": bv_,
                "mask": mask,
            }
        )
    return in_maps


def run_spmd(in_maps, trace=False, **kw):
    nc = _build()
    return run_bass_kernel_spmd(nc, in_maps, core_ids=list(range(NCORE)), trace=trace, **kw)


def gather(results, bo=None):
    y = np.zeros((B, S, D), np.float32)
    for c in range(NCORE):
        # y dram is [128, ST, D] partition-major: y_full[st*128+p] = y[p, st]
        yp = np.asarray(results[c]["y"], np.float32).reshape(128, ST, D)
        y[c // 4] += yp.transpose(1, 0, 2).reshape(S, D)
    if bo is not None:
        y += np.asarray(bo, np.float32)
    return y


def kernel(x, Wq, bq, Wk, bk, Wv, bv, Wo, bo):
    args = [np.asarray(a, np.float32) for a in (x, Wq, bq, Wk, bk, Wv, bv, Wo, bo)]
    in_maps = prep_inputs(*args)
    last_err = None
    for _attempt in range(3):
        try:
            res = run_spmd(in_maps)
            return gather(res.results, bo=args[8])
        except Exception as e:  # transient NRT/axon hiccups: retry
            last_err = e
            import time

            time.sleep(2.0)
    raise last_err

